# revision 1
# baseline (speedup 1.0000x reference)
"""2-layer GCN (GCNConv x2) on 8 Trainium2 NeuronCores.

Strategy (dst-sharded, edge-partitioned by destination):
- Each core owns N/8 destination nodes and the edges pointing at them.
- h~ = dinv * (x @ W1) computed per-shard, AllGathered to a full bf16 table.
- Per-edge messages fetched with dma_gather (4 SWDGE queues round-robin);
  scatter-add done as one-hot-indicator matmuls accumulating in PSUM
  (indicator = is_equal(iota, dstloc) * dinv[dst], built on DVE per chunk).
- Layer 1 accumulates transposed (aggT [hid, dst]) so bias+ReLU ride the
  activation engine per-partition and the block's h2 = out1 @ W2 matmul can
  consume it directly as lhsT; h2~ = dinv * h2 written f32, AllGathered,
  layer 2 repeats the same edge schedule against the h2 table.
"""
import sys
import types

import numpy as np
import ml_dtypes

P = 128
NCORES = 8
GMAX_CHUNKS = 32  # max chunks (128 idxs each) per dma_gather
SB_N = 6  # dst blocks per super-block (one PSUM bank each; 6+1+1 banks)
NQUEUES = 4

_CACHE = {}


# ---------------------------------------------------------------- compat ---
def _install_compat():
    """Patches for this axon/walrus stack (drain waits, per-inst wait caps,
    NTFF shim). Idempotent."""
    if _CACHE.get("compat"):
        return
    import concourse.tile as tile
    import concourse.mybir as mybir

    _ev = [0]

    def _split_inst_waits(ordered):
        for _bb, insts in ordered.items():
            out = []
            for inst in insts:
                si = getattr(inst, "sync_info", None)
                if si is not None and si.on_wait is not None and len(si.on_wait) > 1:
                    waits = list(si.on_wait)
                    excess, keep = waits[:-1], waits[-1:]
                    si.on_wait.clear()
                    for sw in keep:
                        si.on_wait.append(sw)
                    for i in range(0, len(excess), 2):
                        _ev[0] += 1
                        ev = mybir.InstEventSemaphore(
                            name=f"evsplit-{_ev[0]}", ins=[], outs=[]
                        )
                        ev.engine = inst.engine
                        ev.sync_info = mybir.SyncInfo(
                            on_wait=excess[i : i + 2], on_update=[]
                        )
                        out.append(ev)
                out.append(inst)
            insts[:] = out

    orig_lower = tile.TileContext._lower_ordered_insts

    def patched_lower(self, ordered):
        _split_inst_waits(ordered)
        return orig_lower(self, ordered)

    def patched_drain(self, tick_clock, wait_clock):
        sems_alloc = list(self.sems.allocated().values())
        carrier = self.nc.sync.wait_ge(sems_alloc[0], 0)
        wait_clock.add_sem_waits(
            carrier.ins, tile.ScopedClock({None: tick_clock.global_clock})
        )
        waits = list(carrier.ins.sync_info.on_wait)
        carrier.ins.sync_info.on_wait.clear()
        for sw in waits[:2]:
            carrier.ins.sync_info.on_wait.append(sw)
        for i in range(2, len(waits), 2):
            c = self.nc.sync.wait_ge(sems_alloc[0], 0)
            c.ins.sync_info.on_wait.clear()
            for sw in waits[i : i + 2]:
                c.ins.sync_info.on_wait.append(sw)
        self.nc.sync.drain(fusable=False)
        self.nc.all_engine_barrier()
        popped = self.nc._tile_sem_poison_stack.pop()
        assert popped is self._sem_poison
        self.nc.clear_and_free_semaphores(sems_alloc)
        self.nc.all_engine_barrier()

    tile.TileContext._lower_ordered_insts = patched_lower
    tile.TileContext._drain_and_barrier = patched_drain

    # NTFF profile hook shim (missing antenv.axon_hooks in this image)
    _hook = {}
    mod = types.ModuleType("antenv.axon_hooks")
    mod.set_axon_ntff_profile_hook = lambda h: _hook.update(hook=h)
    mod.get_axon_ntff_profile_hook = lambda: _hook.get("hook")
    sys.modules["antenv.axon_hooks"] = mod
    try:
        import antenv

        antenv.axon_hooks = mod
        from trn_agent_boot.trn_boot import _ntff_profile_via_ctypes

        mod.set_axon_ntff_profile_hook(
            _ntff_profile_via_ctypes("/opt/axon/libaxon_pjrt.so")
        )
    except Exception:
        pass
    _CACHE["compat"] = True


# ---------------------------------------------------------- preprocessing ---
class Schedule:
    pass


def _preprocess(n, edge_index):
    """Build the uniform cross-core schedule + per-core data streams."""
    shard = n // NCORES
    nblk = (shard + P - 1) // P
    nbanks = 4
    bank_rows = (n + nbanks - 1) // nbanks
    assert bank_rows <= 32767
    n_sb = (nblk + SB_N - 1) // SB_N

    src = edge_index[0].astype(np.int64)
    dst = edge_index[1].astype(np.int64)
    e = src.shape[0]
    deg = np.bincount(dst, minlength=n).astype(np.float64) + 1.0
    dinv = (1.0 / np.sqrt(deg)).astype(np.float32)

    # append self loops
    loops = np.arange(n, dtype=np.int64)
    src2 = np.concatenate([src, loops])
    dst2 = np.concatenate([dst, loops])

    core = dst2 // shard
    dl = dst2 - core * shard
    blk = dl // P
    dstloc = (dl % P).astype(np.int32)
    bank = src2 // bank_rows
    bidx = (src2 % bank_rows).astype(np.int32)

    # per-core counts per (block, bank)
    cnt = np.zeros((NCORES, nblk, nbanks), np.int64)
    flat = (core * nblk + blk) * nbanks + bank
    bc = np.bincount(flat, minlength=NCORES * nblk * nbanks)
    cnt[...] = bc.reshape(NCORES, nblk, nbanks)
    budget = np.ceil(cnt.max(axis=0) / P).astype(np.int64)  # [nblk, nbanks] chunks

    # schedule: for sb -> for bank -> for blk in sb (budget>0): chunks
    chunk_block = []  # global chunk idx -> block
    chunk_start = []
    chunk_stop = []
    gathers = []  # (col16_off, num_idxs, bank, chunk_off)
    seen_first = np.zeros(nblk, bool)
    # total chunks per block to detect last
    blk_total = budget.sum(axis=1)
    blk_done = np.zeros(nblk, np.int64)
    slot_off = 0
    sb_post = []  # per sb: list of blocks
    for s in range(n_sb):
        blocks = list(range(s * SB_N, min((s + 1) * SB_N, nblk)))
        for k in range(nbanks):
            seg = []  # (block, nchunks)
            for b in blocks:
                if budget[b, k] > 0:
                    seg.append((b, int(budget[b, k])))
            tot = sum(x[1] for x in seg)
            # split into gathers
            coff = len(chunk_block)
            for b, nch in seg:
                for j in range(nch):
                    chunk_block.append(b)
                    chunk_start.append(not seen_first[b])
                    seen_first[b] = True
                    blk_done[b] += 1
                    chunk_stop.append(blk_done[b] == blk_total[b])
            g0 = 0
            while g0 < tot:
                gn = min(GMAX_CHUNKS, tot - g0)
                gathers.append(
                    (slot_off // 16, gn * P, k, coff + g0)
                )
                slot_off += gn * P
                g0 += gn
        sb_post.append(blocks)

    totc = len(chunk_block)
    tot_slots = slot_off
    assert tot_slots == totc * P

    # per-core streams
    idx_stream = np.zeros((NCORES, 16, tot_slots // 16), np.int16)
    dstloc_s = np.full((NCORES, P, totc), -1.0, ml_dtypes.bfloat16)

    sb_arr = blk // SB_N
    order = np.lexsort((blk, bank, sb_arr, core))
    so_core = core[order]
    so_blk = blk[order]
    so_bank = bank[order]
    so_bidx = bidx[order]
    so_dstloc = dstloc[order]

    # walk schedule per core, consuming sorted runs
    ptr = np.searchsorted(so_core, np.arange(NCORES + 1))
    for c in range(NCORES):
        lo, hi = ptr[c], ptr[c + 1]
        cblk = so_blk[lo:hi]
        cbank = so_bank[lo:hi]
        cbidx = so_bidx[lo:hi]
        cdl = so_dstloc[lo:hi]
        csb = cblk // SB_N
        # group boundaries: runs of (sb, bank, blk) in this order already
        key = (csb * nbanks + cbank) * nblk + cblk
        # iterate schedule in same order
        pos = 0
        slot = 0
        idx_flat = np.zeros(tot_slots, np.int16)
        dl_flat = np.full(totc * P, -1.0, np.float32)
        for s in range(n_sb):
            blocks = list(range(s * SB_N, min((s + 1) * SB_N, nblk)))
            for k in range(nbanks):
                for b in blocks:
                    bud = int(budget[b, k])
                    if bud == 0:
                        continue
                    want = (s * nbanks + k) * nblk + b
                    cnt_cb = 0
                    while pos + cnt_cb < hi - lo and key[pos + cnt_cb] == want:
                        cnt_cb += 1
                    nsl = bud * P
                    idx_flat[slot : slot + cnt_cb] = cbidx[pos : pos + cnt_cb]
                    dl_flat[slot : slot + cnt_cb] = cdl[pos : pos + cnt_cb]
                    pos += cnt_cb
                    slot += nsl
        assert pos == hi - lo, (c, pos, hi - lo)
        assert slot == tot_slots
        # wrap: slot i -> idx[i%16, i//16] within each gather's window
        for (c16, nidx, _k, _coff) in gathers:
            sl = slice(c16 * 16, c16 * 16 + nidx)
            seg = idx_flat[sl].reshape(nidx // 16, 16).T  # [16, nidx/16]
            idx_stream[c][:, c16 : c16 + nidx // 16] = seg
        # dstloc layout: chunk C, partition p = slot C*128+p
        dstloc_s[c] = dl_flat.reshape(totc, P).T.astype(ml_dtypes.bfloat16)

    sch = Schedule()
    sch.n, sch.e, sch.shard, sch.nblk, sch.nbanks = n, e, shard, nblk, nbanks
    sch.bank_rows, sch.n_sb, sch.totc = bank_rows, n_sb, totc
    sch.tot_slots = tot_slots
    sch.chunk_block = chunk_block
    sch.chunk_start = chunk_start
    sch.chunk_stop = chunk_stop
    sch.gathers = gathers
    sch.sb_post = sb_post
    sch.budget = budget
    sch.dinv = dinv
    sch.idx_stream = np.tile(idx_stream, (1, 8, 1))  # replicate to 128 partitions
    sch.dstloc_s = dstloc_s
    return sch


# ----------------------------------------------------------------- build ---
def _build(sch, in_dim, hid, out_dim):
    import concourse.mybir as mybir
    import concourse.tile as tile
    from concourse import bacc

    bf16 = mybir.dt.bfloat16
    f32 = mybir.dt.float32
    shard, nblk, nbanks = sch.shard, sch.nblk, sch.nbanks
    totc, n_sb = sch.totc, sch.n_sb
    n = sch.n

    nc = bacc.Bacc(num_swdge_queues=NQUEUES)

    xT = nc.declare_dram_parameter("xT", [in_dim, shard], bf16, isOutput=False)
    idxs = nc.declare_dram_parameter(
        "idxs", [P, sch.tot_slots // 16], mybir.dt.int16, isOutput=False
    )
    dstloc = nc.declare_dram_parameter("dstloc", [P, totc], bf16, isOutput=False)
    iotar_in = nc.declare_dram_parameter("iotar", [P, GMAX_CHUNKS * P], bf16, isOutput=False)
    dinvbc = nc.declare_dram_parameter("dinvbc", [P, nblk * P], f32, isOutput=False)
    dinvb = nc.declare_dram_parameter("dinvb", [P, nblk], f32, isOutput=False)
    w1 = nc.declare_dram_parameter("W1", [in_dim, hid], bf16, isOutput=False)
    b1 = nc.declare_dram_parameter("b1", [hid, 1], f32, isOutput=False)
    w2 = nc.declare_dram_parameter("W2", [hid, out_dim], bf16, isOutput=False)
    b2bc = nc.declare_dram_parameter("b2bc", [P, out_dim], f32, isOutput=False)
    iota_in = nc.declare_dram_parameter("iota", [P, P], bf16, isOutput=False)
    out_ext = nc.declare_dram_parameter("out", [shard, out_dim], f32, isOutput=True)

    hloc = nc.dram_tensor("hloc", [shard, hid], bf16)
    hfull = nc.dram_tensor("hfull", [n, hid], bf16, addr_space="Shared")
    h2loc = nc.dram_tensor("h2loc", [shard, P], bf16)
    h2full = nc.dram_tensor("h2full", [n, P], bf16, addr_space="Shared")

    kin = in_dim // P  # contraction tiles for layer-1 matmul

    with tile.TileContext(nc) as tc:
        with (
            tc.tile_pool(name="const", bufs=1) as cpool,
            tc.tile_pool(name="xload", bufs=2) as xpool,
            tc.tile_pool(name="hb", bufs=2) as hbpool,
            tc.tile_pool(name="idx", bufs=4) as ipool,
            tc.tile_pool(name="gath", bufs=6) as gpool,
            tc.tile_pool(name="sind", bufs=4) as spool,
            tc.tile_pool(name="conv", bufs=8) as vpool,
            tc.tile_pool(name="blk", bufs=3) as bpool,
            tc.tile_pool(name="psh", bufs=1, space="PSUM") as psh,
            tc.tile_pool(name="psagg", bufs=6, space="PSUM") as psagg,
            tc.tile_pool(name="psh2", bufs=1, space="PSUM") as psh2,
        ):
            # one register per distinct gather size, set once
            import contextlib

            regstack = contextlib.ExitStack()
            nidx_vals = sorted({g[1] for g in sch.gathers})
            nreg_map = {}
            for v in nidx_vals:
                r = regstack.enter_context(nc.gpsimd.register(f"nreg_{v}"))
                nc.gpsimd.reg_mov(r, v)
                nreg_map[v] = r
            # ---- constants into SBUF
            iota_sb = cpool.tile([P, P], bf16, tag="iota")
            nc.sync.dma_start(out=iota_sb[:], in_=iota_in[:])
            w1_t = [cpool.tile([P, hid], bf16, tag=f"w1_{k}", name=f"w1t{k}") for k in range(kin)]
            for k in range(kin):
                nc.sync.dma_start(out=w1_t[k][:], in_=w1[k * P : (k + 1) * P, :])
            w2_sb = cpool.tile([hid, out_dim], bf16, tag="w2")
            nc.sync.dma_start(out=w2_sb[:], in_=w2[:])
            b1_sb = cpool.tile([hid, 1], f32, tag="b1")
            nc.sync.dma_start(out=b1_sb[:], in_=b1[:])
            b2_sb = cpool.tile([P, out_dim], f32, tag="b2")
            nc.sync.dma_start(out=b2_sb[:], in_=b2bc[:])
            dinvb_sb = cpool.tile([P, nblk], f32, tag="dinvb")
            nc.sync.dma_start(out=dinvb_sb[:], in_=dinvb[:])
            dstloc_sb = cpool.tile([P, totc], bf16, tag="dstloc")
            nc.sync.dma_start(out=dstloc_sb[:], in_=dstloc[:])
            iotar_sb = cpool.tile([P, GMAX_CHUNKS * P], bf16, tag="iotar")
            nc.sync.dma_start(out=iotar_sb[:], in_=iotar_in[:])

            # ---- h~ = dinv * (x @ W1), shard-local, bf16
            XGRP = 8  # blocks of columns per xT load
            for g0 in range(0, nblk, XGRP):
                g1 = min(g0 + XGRP, nblk)
                c0, c1 = g0 * P, min(g1 * P, shard)
                xt = [
                    xpool.tile([P, c1 - c0], bf16, tag=f"xt{k}", name=f"xt{k}")
                    for k in range(kin)
                ]
                for k in range(kin):
                    nc.sync.dma_start(
                        out=xt[k][:], in_=xT[k * P : (k + 1) * P, c0:c1]
                    )
                for b in range(g0, g1):
                    m = min(P, shard - b * P)
                    hp = psh.tile([P, hid], f32, tag="hps")
                    for k in range(kin):
                        nc.tensor.matmul(
                            out=hp[:m, :],
                            lhsT=xt[k][:, b * P - c0 : b * P - c0 + m],
                            rhs=w1_t[k][:],
                            start=(k == 0),
                            stop=(k == kin - 1),
                        )
                    hsb = hbpool.tile([P, hid], bf16, tag="hsb")
                    nc.scalar.activation(
                        out=hsb[:m, :],
                        in_=hp[:m, :],
                        func=mybir.ActivationFunctionType.Copy,
                        scale=dinvb_sb[:m, b : b + 1],
                    )
                    nc.sync.dma_start(
                        out=hloc[b * P : b * P + m, :], in_=hsb[:m, :]
                    )

            nc.gpsimd.collective_compute(
                "AllGather",
                mybir.AluOpType.bypass,
                ins=[hloc[:]],
                outs=[hfull[:]],
                replica_groups=[list(range(NCORES))],
            )

            # ---- layer pipelines
            def run_layer(layer):
                table = hfull if layer == 1 else h2full
                gq = [0]
                for s in range(n_sb):
                    blocks = sch.sb_post[s]
                    w = P if layer == 1 else out_dim
                    agg_t = {
                        b: psagg.tile([P, w], f32, tag="agg", name=f"agg{s}_{b}")
                        for b in blocks
                    }

                    def slot(b):
                        return agg_t[b][:, :]

                    blocks_set = set(blocks)
                    for (c16, nidx, k, coff) in [
                        g
                        for g in sch.gathers
                        if sch.chunk_block[g[3]] in blocks_set
                    ]:
                        nch = nidx // P
                        it = ipool.tile([P, GMAX_CHUNKS * 8], mybir.dt.int16, tag="it")
                        nc.sync.dma_start(
                            out=it[:, : nidx // 16],
                            in_=idxs[:, c16 : c16 + nidx // 16],
                        )
                        gt = gpool.tile([P, GMAX_CHUNKS, P], bf16, tag="gt")
                        r0 = k * sch.bank_rows
                        r1 = min(r0 + sch.bank_rows, n)
                        nc.gpsimd.dma_gather(
                            out_ap=gt[:, :nch, :],
                            in_ap=table[r0:r1, :],
                            idxs_ap=it[:, : nidx // 16],
                            num_idxs=nidx,
                            num_idxs_reg=nreg_map[nidx],
                            elem_size=P,
                            single_packet=False,
                            queue_num=gq[0] % NQUEUES,
                        )
                        gq[0] += 1
                        # one batched 0/1 indicator build per gather
                        sbig = spool.tile([P, GMAX_CHUNKS, P], bf16, tag="sind")
                        nc.vector.tensor_tensor(
                            out=sbig[:, :nch, :],
                            in0=iotar_sb[:, : nch * P].rearrange(
                                "p (k f) -> p k f", k=nch
                            ),
                            in1=dstloc_sb[:, coff : coff + nch].to_broadcast(
                                [P, nch, P]
                            ),
                            op=mybir.AluOpType.is_equal,
                        )
                        for j in range(nch):
                            C = coff + j
                            b = sch.chunk_block[C]
                            st = sch.chunk_start[C]
                            sp = sch.chunk_stop[C]
                            if layer == 1:
                                nc.tensor.matmul(
                                    out=slot(b),
                                    lhsT=gt[:, j, :],
                                    rhs=sbig[:, j, :],
                                    start=st,
                                    stop=sp,
                                )
                            else:
                                nc.tensor.matmul(
                                    out=slot(b),
                                    lhsT=sbig[:, j, :],
                                    rhs=gt[:, j, :out_dim],
                                    start=st,
                                    stop=sp,
                                )
                    # ---- block epilogue for this super-block
                    for b in blocks:
                        m = min(P, shard - b * P)
                        if layer == 1:
                            dv = bpool.tile([P, P], f32, tag="dv")
                            nc.sync.dma_start(
                                out=dv[:], in_=dinvbc[:, b * P : (b + 1) * P]
                            )
                            t1 = bpool.tile([P, P], bf16, tag="t1")
                            nc.vector.tensor_tensor(
                                out=t1[:],
                                in0=slot(b),
                                in1=dv[:],
                                op=mybir.AluOpType.mult,
                            )
                            o1 = bpool.tile([P, P], bf16, tag="o1")
                            nc.scalar.activation(
                                out=o1[:],
                                in_=t1[:],
                                func=mybir.ActivationFunctionType.Relu,
                                bias=b1_sb[:, :1],
                            )
                            h2p = psh2.tile([P, out_dim], f32, tag="h2p")
                            nc.tensor.matmul(
                                out=h2p[:],
                                lhsT=o1[:],
                                rhs=w2_sb[:],
                                start=True,
                                stop=True,
                            )
                            h2s = bpool.tile([P, P], bf16, tag="h2s")
                            nc.vector.memset(h2s[:, out_dim:], 0.0)
                            nc.scalar.activation(
                                out=h2s[:m, :out_dim],
                                in_=h2p[:m, :],
                                func=mybir.ActivationFunctionType.Copy,
                                scale=dinvb_sb[:m, b : b + 1],
                            )
                            nc.sync.dma_start(
                                out=h2loc[b * P : b * P + m, :], in_=h2s[:m, :]
                            )
                        else:
                            t2 = bpool.tile([P, out_dim], f32, tag="t2")
                            nc.scalar.activation(
                                out=t2[:m, :],
                                in_=slot(b)[:m, :],
                                func=mybir.ActivationFunctionType.Copy,
                                scale=dinvb_sb[:m, b : b + 1],
                            )
                            ob = bpool.tile([P, out_dim], f32, tag="ob")
                            nc.vector.tensor_tensor(
                                out=ob[:m, :],
                                in0=t2[:m, :],
                                in1=b2_sb[:m, :],
                                op=mybir.AluOpType.add,
                            )
                            nc.sync.dma_start(
                                out=out_ext[b * P : b * P + m, :], in_=ob[:m, :]
                            )

            run_layer(1)
            nc.gpsimd.collective_compute(
                "AllGather",
                mybir.AluOpType.bypass,
                ins=[h2loc[:]],
                outs=[h2full[:]],
                replica_groups=[list(range(NCORES))],
            )
            run_layer(2)
            regstack.close()

    nc.compile()
    return nc


# ---------------------------------------------------------------- kernel ---
def _make_in_maps(sch, x, W1, b1v, W2, b2v):
    hid = W1.shape[1]
    out_dim = W2.shape[1]
    shard = sch.shard
    bf = ml_dtypes.bfloat16
    in_maps = []
    w1b = W1.astype(bf)
    w2b = W2.astype(bf)
    b1c = b1v.reshape(hid, 1).astype(np.float32).copy()
    b2c = np.broadcast_to(b2v.astype(np.float32), (P, out_dim)).copy()
    iota = np.broadcast_to(np.arange(P, dtype=np.float32), (P, P)).astype(bf)
    iotar = np.tile(np.arange(P, dtype=np.float32), (P, GMAX_CHUNKS)).astype(bf)
    for c in range(NCORES):
        xs = np.ascontiguousarray(x[c * shard : (c + 1) * shard].astype(bf).T)
        dv = sch.dinv[c * shard : (c + 1) * shard]
        full = np.zeros(sch.nblk * P, np.float32)
        full[:shard] = dv
        dvb = np.ascontiguousarray(full.reshape(sch.nblk, P).T)
        dbc = np.broadcast_to(full, (P, sch.nblk * P)).copy()
        in_maps.append(
            {
                "xT": xs,
                "idxs": sch.idx_stream[c],
                "dstloc": sch.dstloc_s[c],
                "dinvb": dvb,
                "W1": w1b,
                "b1": b1c,
                "W2": w2b,
                "b2bc": b2c,
                "iota": iota,
                "iotar": iotar,
                "dinvbc": dbc,
            }
        )
    return in_maps


def _get_compiled(n, e, edge_index, in_dim, hid, out_dim):
    key = ("nc", n, e)
    if key not in _CACHE:
        sch = _preprocess(n, edge_index)
        _CACHE[("sched", n, e)] = sch
        _CACHE[key] = _build(sch, in_dim, hid, out_dim)
    return _CACHE[("sched", n, e)], _CACHE[key]


def kernel(x, edge_index, W1, b1, W2, b2):
    _install_compat()
    from concourse.bass_utils import run_bass_kernel_spmd

    x = np.asarray(x)
    edge_index = np.asarray(edge_index)
    W1 = np.asarray(W1, np.float32)
    b1v = np.asarray(b1, np.float32)
    W2 = np.asarray(W2, np.float32)
    b2v = np.asarray(b2, np.float32)
    n, in_dim = x.shape
    hid = W1.shape[1]
    out_dim = W2.shape[1]

    sch, nc = _get_compiled(n, edge_index.shape[1], edge_index, in_dim, hid, out_dim)
    in_maps = _make_in_maps(sch, x, W1, b1v, W2, b2v)
    import os

    trace = bool(os.environ.get("GCN_TRACE"))
    res = run_bass_kernel_spmd(
        nc, in_maps, core_ids=list(range(NCORES)), trace=trace
    )
    global LAST_EXEC_NS
    LAST_EXEC_NS = res.exec_time_ns
    return np.concatenate([res.results[c]["out"] for c in range(NCORES)], axis=0)


LAST_EXEC_NS = None



# revision 8
# speedup vs baseline: 2.4414x; 2.4414x over previous
"""2-layer GCN (GCNConv x2) on 8 Trainium2 NeuronCores.

Strategy (dst-sharded, edge-partitioned by destination; Q7-descgen-optimized):
- Each core owns N/8 destination nodes and the edges pointing at them.
- Table banks: bank q = concat over cores of quarter-q of their shard
  (block-aligned quarters), AllGathered per-bank so gathers unblock early.
- h~ = dinv * (x @ W1) computed per-shard bf16, 4 bank-wise AllGathers.
- Per (super-block of 6 dst blocks, bank): edges packed contiguously in
  block order into 128-slot chunks (straddling block boundaries); one
  dma_gather per segment on queue=bank (4 SWDGE queue contexts, balanced).
  Scatter-add via is_equal-indicator matmuls accumulating in PSUM.
- Artificial self-loops are NOT gathered: per block one identity matmul
  accumulates the dinv-scaled own-row table entries (32KB sequential load).
- Per-core padding slots sit at gather tails as -1 indices, which the Q7
  SWDGE kernel trims (descgen cost ~= actual per-core edge count).
- Layer 1 accumulates transposed (aggT [feat, dst]) so bias+ReLU ride the
  activation engine and out1 feeds h2 = out1 @ W2 directly as lhsT;
  h2~ = dinv * h2 goes out bf16 through 4 more bank-wise AllGathers placed
  as their quarters complete, so layer 2's gathers pipeline behind layer 1.
"""
import sys
import types

import numpy as np
import ml_dtypes

P = 128
NCORES = 8
GMAX = 32  # max chunks (128 idxs each) per dma_gather
SB_N = 6  # dst blocks per super-block (one PSUM bank each; 6+1+1 banks)
NQUEUES = 4
XGRP = 8
GBUFS = 7
SBUFS = 5
AG2_LAG = 2  # super-blocks between a quarter finishing and its AG2 trigger

_CACHE = {}


# ---------------------------------------------------------------- compat ---
def _install_compat():
    """Patches for this axon/walrus stack (drain waits, per-inst wait caps,
    NTFF shim). Idempotent."""
    if _CACHE.get("compat"):
        return
    import concourse.tile as tile
    import concourse.mybir as mybir

    _ev = [0]

    def _split_inst_waits(ordered):
        for _bb, insts in ordered.items():
            out = []
            for inst in insts:
                si = getattr(inst, "sync_info", None)
                if si is not None and si.on_wait is not None and len(si.on_wait) > 1:
                    waits = list(si.on_wait)
                    excess, keep = waits[:-1], waits[-1:]
                    si.on_wait.clear()
                    for sw in keep:
                        si.on_wait.append(sw)
                    for i in range(0, len(excess), 2):
                        _ev[0] += 1
                        ev = mybir.InstEventSemaphore(
                            name=f"evsplit-{_ev[0]}", ins=[], outs=[]
                        )
                        ev.engine = inst.engine
                        ev.sync_info = mybir.SyncInfo(
                            on_wait=excess[i : i + 2], on_update=[]
                        )
                        out.append(ev)
                out.append(inst)
            insts[:] = out

    orig_lower = tile.TileContext._lower_ordered_insts

    def patched_lower(self, ordered):
        _split_inst_waits(ordered)
        return orig_lower(self, ordered)

    def patched_drain(self, tick_clock, wait_clock):
        sems_alloc = list(self.sems.allocated().values())
        carrier = self.nc.sync.wait_ge(sems_alloc[0], 0)
        wait_clock.add_sem_waits(
            carrier.ins, tile.ScopedClock({None: tick_clock.global_clock})
        )
        waits = list(carrier.ins.sync_info.on_wait)
        carrier.ins.sync_info.on_wait.clear()
        for sw in waits[:2]:
            carrier.ins.sync_info.on_wait.append(sw)
        for i in range(2, len(waits), 2):
            c = self.nc.sync.wait_ge(sems_alloc[0], 0)
            c.ins.sync_info.on_wait.clear()
            for sw in waits[i : i + 2]:
                c.ins.sync_info.on_wait.append(sw)
        self.nc.sync.drain(fusable=False)
        self.nc.all_engine_barrier()
        popped = self.nc._tile_sem_poison_stack.pop()
        assert popped is self._sem_poison
        self.nc.clear_and_free_semaphores(sems_alloc)
        self.nc.all_engine_barrier()

    tile.TileContext._lower_ordered_insts = patched_lower
    tile.TileContext._drain_and_barrier = patched_drain

    # NTFF profile hook shim (missing antenv.axon_hooks in this image)
    _hook = {}
    mod = types.ModuleType("antenv.axon_hooks")
    mod.set_axon_ntff_profile_hook = lambda h: _hook.update(hook=h)
    mod.get_axon_ntff_profile_hook = lambda: _hook.get("hook")
    sys.modules["antenv.axon_hooks"] = mod
    try:
        import antenv

        antenv.axon_hooks = mod
        from trn_agent_boot.trn_boot import _ntff_profile_via_ctypes

        mod.set_axon_ntff_profile_hook(
            _ntff_profile_via_ctypes("/opt/axon/libaxon_pjrt.so")
        )
    except Exception:
        pass
    _CACHE["compat"] = True


# ---------------------------------------------------------- preprocessing ---
class Schedule:
    pass


def _quarter_bounds(nblk):
    base, rem = nblk // 4, nblk % 4
    sizes = [base + (1 if i < rem else 0) for i in range(4)]
    starts = np.cumsum([0] + sizes)
    return [(int(starts[i]), int(starts[i + 1])) for i in range(4)]


def _preprocess(n, edge_index):
    src = np.asarray(edge_index[0], np.int64)
    dst = np.asarray(edge_index[1], np.int64)
    e = src.shape[0]
    shard = n // NCORES
    nblk = (shard + P - 1) // P
    n_sb = (nblk + SB_N - 1) // SB_N
    qb = _quarter_bounds(nblk)
    qrow_start = [b0 * P for b0, b1 in qb]
    qrows = [min(b1 * P, shard) - b0 * P for b0, b1 in qb]
    bank_n = [NCORES * r for r in qrows]
    assert all(b <= 32767 for b in bank_n)

    deg = np.bincount(dst, minlength=n).astype(np.float64) + 1.0
    dinv = (1.0 / np.sqrt(deg)).astype(np.float32)

    core_d = dst // shard
    dl = dst - core_d * shard
    blk = dl // P
    dstloc = (dl % P).astype(np.int64)
    sb = blk // SB_N
    core_s = src // shard
    off = src - core_s * shard
    sblk = off // P
    qb_arr = np.zeros(nblk, np.int64)
    for q, (b0, b1) in enumerate(qb):
        qb_arr[b0:b1] = q
    bank = qb_arr[sblk]
    bidx = (
        core_s * np.array(qrows)[bank] + (off - np.array(qrow_start)[bank])
    ).astype(np.int64)

    order = np.lexsort((blk, bank, sb, core_d))
    s_core = core_d[order]
    s_sb = sb[order]
    s_bank = bank[order]
    s_blk = blk[order]
    s_bidx = bidx[order]
    s_dstloc = dstloc[order]

    key = ((s_core * n_sb + s_sb) * 4 + s_bank) * nblk + s_blk
    cnt = np.bincount(key, minlength=NCORES * n_sb * 4 * nblk).reshape(
        NCORES, n_sb, 4, nblk
    )

    gathers = []
    slots = []
    last_slot_of_block = {}
    chunk_gid = 0
    slot_gid = 0
    budget_tab = np.zeros((n_sb, 4), np.int64)
    for s in range(n_sb):
        blocks = list(range(s * SB_N, min((s + 1) * SB_N, nblk)))
        for k in range(4):
            percore = cnt[:, s, k, :][:, blocks]
            cum = np.cumsum(percore, axis=1)
            budget = max(int(np.ceil(cum[:, -1].max() / P)), 1)
            budget_tab[s, k] = budget
            lo = np.min(cum - percore, axis=0)
            hi = np.max(cum, axis=0)
            g0 = 0
            while g0 < budget:
                gn = min(GMAX, budget - g0)
                gi = len(gathers)
                gslot0 = slot_gid
                for j in range(g0, g0 + gn):
                    c_lo, c_hi = j * P, (j + 1) * P
                    for bi, b in enumerate(blocks):
                        if hi[bi] > c_lo and lo[bi] < c_hi:
                            slots.append(
                                dict(
                                    g=gi,
                                    cl=j - g0,
                                    blk=b,
                                    sb=s,
                                    bank=k,
                                    sl_local=slot_gid - gslot0,
                                    chunk_gid=chunk_gid + j,
                                )
                            )
                            last_slot_of_block[(s, b)] = slot_gid
                            slot_gid += 1
                gathers.append(
                    dict(
                        gi=gi,
                        sb=s,
                        bank=k,
                        chunk0=chunk_gid + g0,
                        nch=gn,
                        nidx=gn * P,
                        slot0=gslot0,
                        nslots=slot_gid - gslot0,
                        c16=(chunk_gid + g0) * P // 16,
                    )
                )
                g0 += gn
            chunk_gid += budget
    totc = chunk_gid
    nslots = slot_gid
    tot_slots = totc * P
    for i, sl in enumerate(slots):
        sl["stop"] = last_slot_of_block[(sl["sb"], sl["blk"])] == i
    slotmax = max(g["nslots"] for g in gathers)

    seg_key = (s_core * n_sb + s_sb) * 4 + s_bank
    seg_ptr = np.searchsorted(seg_key, np.arange(NCORES * n_sb * 4 + 1))
    seg_chunk0 = {}
    cg = 0
    for s in range(n_sb):
        for k in range(4):
            seg_chunk0[(s, k)] = cg
            cg += int(budget_tab[s, k])

    idx_flat = np.zeros((NCORES, tot_slots), np.int16)
    dstloc_s = np.full((NCORES, P, nslots), -1.0, np.float32)
    for c in range(NCORES):
        arr = idx_flat[c]
        for s in range(n_sb):
            for k in range(4):
                p0 = seg_ptr[(c * n_sb + s) * 4 + k]
                p1 = seg_ptr[(c * n_sb + s) * 4 + k + 1]
                cnt_c = p1 - p0
                base = seg_chunk0[(s, k)] * P
                arr[base : base + cnt_c] = s_bidx[p0:p1].astype(np.int16)
                # pads stay 0 (gather row 0, indicator -1 masks them out).
                # NOTE: trailing -1 trimming is NOT safe here: the decode-side
                # ring reservation uses num_idxs_reg (static) while gen_descs
                # would push the trimmed count -> ring bookkeeping divergence.

    seg_id = (s_core * n_sb + s_sb) * 4 + s_bank
    pos_in_seg = np.arange(e) - seg_ptr[seg_id]
    seg_chunk0_arr = np.zeros(NCORES * n_sb * 4, np.int64)
    for s in range(n_sb):
        for k in range(4):
            for c in range(NCORES):
                seg_chunk0_arr[(c * n_sb + s) * 4 + k] = seg_chunk0[(s, k)]
    chunk_of_edge = seg_chunk0_arr[seg_id] + pos_in_seg // P
    part_of_edge = pos_in_seg % P
    slot_lut = np.full((totc, nblk), -1, np.int64)
    for i, sl in enumerate(slots):
        slot_lut[sl["chunk_gid"], sl["blk"]] = i
    slot_of_edge = slot_lut[chunk_of_edge, s_blk]
    assert (slot_of_edge >= 0).all()
    dstloc_s[s_core, part_of_edge, slot_of_edge] = s_dstloc

    # wrap idx stream: slot i -> [lane i%16, col i//16], replicate to 128 parts
    idx_stream = np.ascontiguousarray(
        idx_flat.reshape(NCORES, tot_slots // 16, 16).transpose(0, 2, 1)
    )
    idx_stream = np.tile(idx_stream, (1, 8, 1))

    sch = Schedule()
    sch.n, sch.e, sch.shard, sch.nblk, sch.n_sb = n, e, shard, nblk, n_sb
    sch.qb, sch.qrow_start, sch.qrows, sch.bank_n = qb, qrow_start, qrows, bank_n
    sch.dinv = dinv
    sch.gathers = gathers
    sch.slots = slots
    sch.totc, sch.nslots, sch.tot_slots = totc, nslots, tot_slots
    sch.slotmax = slotmax
    sch.idx_stream = idx_stream
    sch.idx_flat = idx_flat
    sch.dstloc_s = dstloc_s.astype(ml_dtypes.bfloat16)
    return sch


# ----------------------------------------------------------------- build ---
def _build(sch, in_dim, hid, out_dim):
    import concourse.mybir as mybir
    import concourse.tile as tile
    from concourse import bacc

    bf16 = mybir.dt.bfloat16
    f32 = mybir.dt.float32
    shard, nblk, n_sb = sch.shard, sch.nblk, sch.n_sb
    slotmax = sch.slotmax
    qb = sch.qb

    nc = bacc.Bacc(num_swdge_queues=NQUEUES)

    xT = nc.declare_dram_parameter("xT", [in_dim, shard], bf16, isOutput=False)
    idxs = nc.declare_dram_parameter(
        "idxs", [P, sch.tot_slots // 16], mybir.dt.int16, isOutput=False
    )
    dstloc = nc.declare_dram_parameter("dstloc", [P, sch.nslots], bf16, isOutput=False)
    iotar_in = nc.declare_dram_parameter("iotar", [P, slotmax * P], bf16, isOutput=False)
    dinvbc = nc.declare_dram_parameter("dinvbc", [P, nblk * P], f32, isOutput=False)
    dinvb = nc.declare_dram_parameter("dinvb", [P, nblk], f32, isOutput=False)
    w1 = nc.declare_dram_parameter("W1", [in_dim, hid], bf16, isOutput=False)
    b1 = nc.declare_dram_parameter("b1", [hid, 1], f32, isOutput=False)
    w2 = nc.declare_dram_parameter("W2", [hid, out_dim], bf16, isOutput=False)
    b2bc = nc.declare_dram_parameter("b2bc", [P, out_dim], f32, isOutput=False)
    ident_in = nc.declare_dram_parameter("ident", [P, P], bf16, isOutput=False)
    out_ext = nc.declare_dram_parameter("out", [shard, out_dim], f32, isOutput=True)

    hloc_q = [
        nc.dram_tensor(f"hloc{q}", [sch.qrows[q], P], bf16) for q in range(4)
    ]
    hbank = [
        nc.dram_tensor(f"hbank{q}", [sch.bank_n[q], P], bf16, addr_space="Shared")
        for q in range(4)
    ]
    h2loc_q = [
        nc.dram_tensor(f"h2loc{q}", [sch.qrows[q], P], bf16) for q in range(4)
    ]
    h2bank = [
        nc.dram_tensor(f"h2bank{q}", [sch.bank_n[q], P], bf16, addr_space="Shared")
        for q in range(4)
    ]

    kin = in_dim // P
    # quarter of a block
    def quarter_of(b):
        for q, (b0, b1) in enumerate(qb):
            if b0 <= b < b1:
                return q
        raise AssertionError

    # gathers grouped per (sb, bank)
    seg_gathers = {}
    for g in sch.gathers:
        seg_gathers.setdefault((g["sb"], g["bank"]), []).append(g)
    # slots grouped per gather
    g_slots = {}
    for sl in sch.slots:
        g_slots.setdefault(sl["g"], []).append(sl)

    # AG2 trigger placement: quarter q's last sb + AG2_LAG
    ag2_at_sb = {}
    for q in range(4):
        sq_end = (qb[q][1] - 1) // SB_N
        ag2_at_sb.setdefault(min(sq_end + AG2_LAG, n_sb - 1) if q < 3 else n_sb - 1, []).append(q)

    with tile.TileContext(nc) as tc:
        with (
            tc.tile_pool(name="const", bufs=1) as cpool,
            tc.tile_pool(name="xload", bufs=2) as xpool,
            tc.tile_pool(name="hb", bufs=2) as hbpool,
            tc.tile_pool(name="idx", bufs=8) as ipool,
            tc.tile_pool(name="gath", bufs=GBUFS) as gpool,
            tc.tile_pool(name="sind", bufs=SBUFS) as spool,
            tc.tile_pool(name="rl", bufs=4) as rpool,
            tc.tile_pool(name="blk", bufs=3) as bpool,
            tc.tile_pool(name="psh", bufs=1, space="PSUM") as psh,
            tc.tile_pool(name="psagg", bufs=6, space="PSUM") as psagg,
            tc.tile_pool(name="psh2", bufs=1, space="PSUM") as psh2,
        ):
            import contextlib

            regstack = contextlib.ExitStack()
            nidx_vals = sorted({g["nidx"] for g in sch.gathers})
            nreg_map = {}
            for v in nidx_vals:
                r = regstack.enter_context(nc.gpsimd.register(f"nreg_{v}"))
                nc.gpsimd.reg_mov(r, v)
                nreg_map[v] = r

            # ---- constants into SBUF
            ident_sb = cpool.tile([P, P], bf16, tag="ident")
            nc.sync.dma_start(out=ident_sb[:], in_=ident_in[:])
            w1_t = [
                cpool.tile([P, hid], bf16, tag=f"w1_{k}", name=f"w1t{k}")
                for k in range(kin)
            ]
            for k in range(kin):
                nc.sync.dma_start(out=w1_t[k][:], in_=w1[k * P : (k + 1) * P, :])
            w2_sb = cpool.tile([hid, out_dim], bf16, tag="w2")
            nc.sync.dma_start(out=w2_sb[:], in_=w2[:])
            b1_sb = cpool.tile([hid, 1], f32, tag="b1")
            nc.sync.dma_start(out=b1_sb[:], in_=b1[:])
            b2_sb = cpool.tile([P, out_dim], f32, tag="b2")
            nc.sync.dma_start(out=b2_sb[:], in_=b2bc[:])
            dinvb_sb = cpool.tile([P, nblk], f32, tag="dinvb")
            nc.sync.dma_start(out=dinvb_sb[:], in_=dinvb[:])
            dstloc_sb = cpool.tile([P, sch.nslots], bf16, tag="dstloc")
            nc.sync.dma_start(out=dstloc_sb[:], in_=dstloc[:])
            iotar_sb = cpool.tile([P, slotmax * P], bf16, tag="iotar")
            nc.sync.dma_start(out=iotar_sb[:], in_=iotar_in[:])

            # ---- zero-init gather buffers (trimmed gathers may leave chunks
            # unwritten; stale garbage x 0-indicator must not be NaN)
            for _ in range(GBUFS):
                gz = gpool.tile([P, GMAX, P], bf16, tag="gt")
                nc.vector.memset(gz[:], 0.0)

            # ---- h~ = dinv * (x @ W1), shard-local, bf16; AG per quarter
            for g0 in range(0, nblk, XGRP):
                g1 = min(g0 + XGRP, nblk)
                c0, c1 = g0 * P, min(g1 * P, shard)
                xt = [
                    xpool.tile([P, c1 - c0], bf16, tag=f"xt{k}", name=f"xt{k}")
                    for k in range(kin)
                ]
                for k in range(kin):
                    nc.sync.dma_start(out=xt[k][:], in_=xT[k * P : (k + 1) * P, c0:c1])
                for b in range(g0, g1):
                    m = min(P, shard - b * P)
                    hp = psh.tile([P, hid], f32, tag="hps")
                    for k in range(kin):
                        nc.tensor.matmul(
                            out=hp[:m, :],
                            lhsT=xt[k][:, b * P - c0 : b * P - c0 + m],
                            rhs=w1_t[k][:],
                            start=(k == 0),
                            stop=(k == kin - 1),
                        )
                    hsb = hbpool.tile([P, hid], bf16, tag="hsb")
                    nc.scalar.activation(
                        out=hsb[:m, :],
                        in_=hp[:m, :],
                        func=mybir.ActivationFunctionType.Copy,
                        scale=dinvb_sb[:m, b : b + 1],
                    )
                    q = quarter_of(b)
                    r0 = b * P - sch.qrow_start[q]
                    nc.sync.dma_start(out=hloc_q[q][r0 : r0 + m, :], in_=hsb[:m, :])
                    if b == qb[q][1] - 1:
                        nc.gpsimd.collective_compute(
                            "AllGather",
                            mybir.AluOpType.bypass,
                            ins=[hloc_q[q][:]],
                            outs=[hbank[q][:]],
                            replica_groups=[list(range(NCORES))],
                        )

            # ---- layer pipelines
            def run_layer(layer):
                table = hbank if layer == 1 else h2bank
                loc_q = hloc_q if layer == 1 else h2loc_q
                w = P if layer == 1 else out_dim
                for s in range(n_sb):
                    blocks = list(range(s * SB_N, min((s + 1) * SB_N, nblk)))
                    agg_t = {
                        b: psagg.tile([P, w], f32, tag="agg", name=f"agg{layer}_{s}_{b}")
                        for b in blocks
                    }
                    # self-loop contribution: identity matmul from own rows
                    for b in blocks:
                        m = min(P, shard - b * P)
                        q = quarter_of(b)
                        r0 = b * P - sch.qrow_start[q]
                        rl = rpool.tile([P, P], bf16, tag="rl")
                        nc.sync.dma_start(
                            out=rl[:m, :], in_=loc_q[q][r0 : r0 + m, :]
                        )
                        if layer == 1:
                            nc.tensor.matmul(
                                out=agg_t[b][:, :],
                                lhsT=rl[:m, :],
                                rhs=ident_sb[:m, :],
                                start=True,
                                stop=False,
                            )
                        else:
                            nc.tensor.matmul(
                                out=agg_t[b][:, :],
                                lhsT=ident_sb[:m, :],
                                rhs=rl[:m, :out_dim],
                                start=True,
                                stop=False,
                            )
                    for k in range(4):
                        for g in seg_gathers.get((s, k), []):
                            nidx, nch = g["nidx"], g["nch"]
                            it = ipool.tile([P, GMAX * 8], mybir.dt.int16, tag="it")
                            nc.sync.dma_start(
                                out=it[:, : nidx // 16],
                                in_=idxs[:, g["c16"] : g["c16"] + nidx // 16],
                            )
                            gt = gpool.tile([P, GMAX, P], bf16, tag="gt")
                            nc.gpsimd.dma_gather(
                                out_ap=gt[:, :nch, :],
                                in_ap=table[k][0 : sch.bank_n[k], :],
                                idxs_ap=it[:, : nidx // 16],
                                num_idxs=nidx,
                                num_idxs_reg=nreg_map[nidx],
                                elem_size=P,
                                single_packet=False,
                                queue_num=k,
                            )
                            nsl = g["nslots"]
                            sbig = spool.tile([P, slotmax, P], bf16, tag="sind")
                            nc.vector.tensor_tensor(
                                out=sbig[:, :nsl, :],
                                in0=iotar_sb[:, : nsl * P].rearrange(
                                    "p (k f) -> p k f", k=nsl
                                ),
                                in1=dstloc_sb[
                                    :, g["slot0"] : g["slot0"] + nsl
                                ].to_broadcast([P, nsl, P]),
                                op=mybir.AluOpType.is_equal,
                            )
                            for sl in g_slots.get(g["gi"], []):
                                if layer == 1:
                                    nc.tensor.matmul(
                                        out=agg_t[sl["blk"]][:, :],
                                        lhsT=gt[:, sl["cl"], :],
                                        rhs=sbig[:, sl["sl_local"], :],
                                        start=False,
                                        stop=sl["stop"],
                                    )
                                else:
                                    nc.tensor.matmul(
                                        out=agg_t[sl["blk"]][:, :],
                                        lhsT=sbig[:, sl["sl_local"], :],
                                        rhs=gt[:, sl["cl"], :out_dim],
                                        start=False,
                                        stop=sl["stop"],
                                    )
                    # ---- block epilogues
                    for b in blocks:
                        m = min(P, shard - b * P)
                        if layer == 1:
                            dv = bpool.tile([P, P], f32, tag="dv")
                            nc.sync.dma_start(
                                out=dv[:], in_=dinvbc[:, b * P : (b + 1) * P]
                            )
                            t1 = bpool.tile([P, P], bf16, tag="t1")
                            nc.vector.tensor_tensor(
                                out=t1[:],
                                in0=agg_t[b][:, :],
                                in1=dv[:],
                                op=mybir.AluOpType.mult,
                            )
                            o1 = bpool.tile([P, P], bf16, tag="o1")
                            nc.scalar.activation(
                                out=o1[:],
                                in_=t1[:],
                                func=mybir.ActivationFunctionType.Relu,
                                bias=b1_sb[:, :1],
                            )
                            h2p = psh2.tile([P, out_dim], f32, tag="h2p")
                            nc.tensor.matmul(
                                out=h2p[:],
                                lhsT=o1[:],
                                rhs=w2_sb[:],
                                start=True,
                                stop=True,
                            )
                            h2s = bpool.tile([P, P], bf16, tag="h2s")
                            nc.scalar.activation(
                                out=h2s[:m, :out_dim],
                                in_=h2p[:m, :],
                                func=mybir.ActivationFunctionType.Copy,
                                scale=dinvb_sb[:m, b : b + 1],
                            )
                            q = quarter_of(b)
                            r0 = b * P - sch.qrow_start[q]
                            nc.sync.dma_start(
                                out=h2loc_q[q][r0 : r0 + m, 0:out_dim],
                                in_=h2s[:m, :out_dim],
                            )
                        else:
                            t2 = bpool.tile([P, out_dim], f32, tag="t2")
                            nc.scalar.activation(
                                out=t2[:m, :],
                                in_=agg_t[b][:m, :],
                                func=mybir.ActivationFunctionType.Copy,
                                scale=dinvb_sb[:m, b : b + 1],
                            )
                            ob = bpool.tile([P, out_dim], f32, tag="ob")
                            nc.vector.tensor_tensor(
                                out=ob[:m, :],
                                in0=t2[:m, :],
                                in1=b2_sb[:m, :],
                                op=mybir.AluOpType.add,
                            )
                            nc.sync.dma_start(
                                out=out_ext[b * P : b * P + m, :], in_=ob[:m, :]
                            )
                    # ---- bank-wise AG2 triggers as quarters complete
                    if layer == 1:
                        for q in ag2_at_sb.get(s, []):
                            nc.gpsimd.collective_compute(
                                "AllGather",
                                mybir.AluOpType.bypass,
                                ins=[h2loc_q[q][:]],
                                outs=[h2bank[q][:]],
                                replica_groups=[list(range(NCORES))],
                            )

            run_layer(1)
            run_layer(2)
            regstack.close()

    nc.compile()
    return nc


# ---------------------------------------------------------------- kernel ---
def _make_in_maps(sch, x, W1, b1v, W2, b2v):
    hid = W1.shape[1]
    out_dim = W2.shape[1]
    shard, nblk = sch.shard, sch.nblk
    bf = ml_dtypes.bfloat16
    in_maps = []
    w1b = W1.astype(bf)
    w2b = W2.astype(bf)
    b1c = b1v.reshape(hid, 1).astype(np.float32).copy()
    b2c = np.broadcast_to(b2v.astype(np.float32), (P, out_dim)).copy()
    ident = np.eye(P, dtype=np.float32).astype(bf)
    iotar = np.tile(np.arange(P, dtype=np.float32), (P, sch.slotmax)).astype(bf)
    for c in range(NCORES):
        xs = np.ascontiguousarray(x[c * shard : (c + 1) * shard].astype(bf).T)
        dv = sch.dinv[c * shard : (c + 1) * shard]
        full = np.zeros(nblk * P, np.float32)
        full[:shard] = dv
        dvb = np.ascontiguousarray(full.reshape(nblk, P).T)
        dbc = np.broadcast_to(full, (P, nblk * P)).copy()
        in_maps.append(
            {
                "xT": xs,
                "idxs": sch.idx_stream[c],
                "dstloc": sch.dstloc_s[c],
                "dinvb": dvb,
                "W1": w1b,
                "b1": b1c,
                "W2": w2b,
                "b2bc": b2c,
                "ident": ident,
                "iotar": iotar,
                "dinvbc": dbc,
            }
        )
    return in_maps


def _get_compiled(n, e, edge_index, in_dim, hid, out_dim):
    key = ("nc", n, e)
    if key not in _CACHE:
        sch = _preprocess(n, edge_index)
        _CACHE[("sched", n, e)] = sch
        _CACHE[key] = _build(sch, in_dim, hid, out_dim)
    return _CACHE[("sched", n, e)], _CACHE[key]


def kernel(x, edge_index, W1, b1, W2, b2):
    _install_compat()
    from concourse.bass_utils import run_bass_kernel_spmd

    x = np.asarray(x)
    edge_index = np.asarray(edge_index)
    W1 = np.asarray(W1, np.float32)
    b1v = np.asarray(b1, np.float32)
    W2 = np.asarray(W2, np.float32)
    b2v = np.asarray(b2, np.float32)
    n, in_dim = x.shape
    hid = W1.shape[1]
    out_dim = W2.shape[1]

    sch, nc = _get_compiled(n, edge_index.shape[1], edge_index, in_dim, hid, out_dim)
    in_maps = _make_in_maps(sch, x, W1, b1v, W2, b2v)
    import os

    trace = bool(os.environ.get("GCN_TRACE"))
    res = run_bass_kernel_spmd(
        nc, in_maps, core_ids=list(range(NCORES)), trace=trace
    )
    global LAST_EXEC_NS
    LAST_EXEC_NS = res.exec_time_ns
    return np.concatenate([res.results[c]["out"] for c in range(NCORES)], axis=0)


LAST_EXEC_NS = None
